# revision 8
# baseline (speedup 1.0000x reference)
"""3-layer GCN encoder (GCNConv+BN+ReLU x3) on 8 Trainium2 NeuronCores.

Strategy (graph/data-parallel over destination nodes), v2:
  - Nodes padded 50000 -> 50176 = 8 * 6272; core c owns dst rows
    [c*6272, (c+1)*6272) = 49 blocks of 128.
  - Per layer l: each core computes its shard of H = X @ W_l channel-major
    on the PE (fp32r), fuses the dinv = rsqrt(deg) source scaling into the
    PSUM->SBUF copy producing a bf16 table shard, transposes it to
    node-major, and AllGathers the full 50176 x 64 bf16 table (6.4 MB).
  - Message passing: edges sorted by dst block and split by src PARITY;
    dma_gather fetches 256B row-PAIRS (bf16, idx = src>>1, int16-safe);
    per 128-edge tile a one-hot [slot -> dstrel] bf16 matmul segment-sums
    messages into a [64ch x 128dst] PSUM accumulator, with the matmul lhsT
    picking the even/odd 64-column half of the gathered pair. Self loops
    are folded in as one identity matmul per block from the resident
    node-major table shard (no edges, no gather traffic). The result is
    scaled by dinv[dst].
  - BatchNorm: per-channel sum / sumsq via ACT accum_out, AllReduce,
    then one fused Relu(S*A + B) activation. Conv biases are absorbed by
    BN mean subtraction. Padding rows are killed via deg_pad = 1e30.
  - Sync cleanup: same-engine semaphore waits are dropped (engines execute
    in order) and per-engine monotone waits are deduplicated, minimizing
    waitsplit NoOps on the DVE/PE sequencers.
  - Host side does integer index preprocessing only (sort/partition/pad,
    degree counting, layout transposes); all FP math runs on device.
"""
import sys
sys.path.insert(0, "/opt/trn_rl_repo")
import numpy as np

import concourse.bass as bass
import concourse.mybir as mybir
import concourse.tile as tile
from concourse import library_config
from concourse.library_overlay import lower_extended_insts
from concourse.masks import make_identity

N = 50000
NPAD = 50176
NCORES = 8
SHARD = NPAD // NCORES          # 6272
NB = SHARD // 128               # 49 blocks per core
PAIRS = NPAD // 2               # 25088 pair rows (int16-safe)
IN_C = 128
HID = 64
BN_EPS = 1e-5
F32 = mybir.dt.float32
F32R = mybir.dt.float32r
BF16 = mybir.dt.bfloat16
I16 = mybir.dt.int16
NQ = 4                          # swdge queues (ucode max)


_CLEAN = [False]

_ENGINE_OPS = {
    "InstTensorScalarPtr", "InstTensorScalar", "InstTensorTensor",
    "InstTensorCopy", "InstMatmult", "InstActivation", "InstMemset",
    "InstReciprocal", "InstTensorReduce", "InstIota",
    "InstTensorScalarAffineSelect",
}


def _clean_waits(nc):
    """Two semantics-preserving wait removals (engines execute in order):
    1. drop waits on the instruction's own engine-completion semaphore;
    2. per engine, drop sem-ge waits made redundant by an earlier, larger
       wait on the same semaphore (transitive via program order).
    Only applied to monotone (inc-only after first wait) semaphores."""
    insts = []
    for func in nc.m.functions:
        for bb in func.blocks:
            insts.extend(bb.instructions)

    first_wait = {}
    unsafe = set()
    for i, inst in enumerate(insts):
        si = inst.sync_info
        if si is None:
            continue
        for w in si.on_wait:
            if w.ant_name not in first_wait:
                first_wait[w.ant_name] = i
    for i, inst in enumerate(insts):
        si = inst.sync_info
        if si is None:
            continue
        for u in si.on_update:
            if u.update_mode != "sem-inc" and first_wait.get(u.ant_name, 1 << 60) < i:
                unsafe.add(u.ant_name)

    dropped_same = dropped_dup = 0
    watermark = {}
    for inst in insts:
        si = inst.sync_info
        if si is None or not si.on_wait:
            continue
        eng = inst.engine.value if hasattr(inst.engine, "value") else str(inst.engine)
        is_eng_op = type(inst).__name__ in _ENGINE_OPS
        keep = []
        for w in si.on_wait:
            if w.wait_mode != "sem-ge-imm" or w.ant_name in unsafe:
                keep.append(w)
                continue
            prefix = w.ant_name.rsplit("_", 1)[0]
            if is_eng_op and prefix == eng:
                dropped_same += 1
                continue
            key = (eng, w.ant_name)
            if watermark.get(key, -1) >= w.wait_value:
                dropped_dup += 1
                continue
            watermark[key] = w.wait_value
            keep.append(w)
        inst.sync_info = mybir.SyncInfo(on_wait=keep, on_update=list(si.on_update))
    return dropped_same, dropped_dup


def _split_multi_waits(nc, cap=1):
    """walrus in this toolchain accepts one sync wait per instruction;
    hoist extras onto standalone same-engine NOPs."""
    ctr = 0
    for func in nc.m.functions:
        for bb in func.blocks:
            new_insts = []
            for inst in bb.instructions:
                si = inst.sync_info
                if si is not None and len(si.on_wait) > cap:
                    waits = list(si.on_wait)
                    for w in waits[:-cap]:
                        ctr += 1
                        new_insts.append(mybir.InstNoOp(
                            name=f"waitsplit-{ctr}-{inst.name}",
                            sync_info=mybir.SyncInfo(on_wait=[w], on_update=[]),
                            bass_nofuse=True,
                            engine=inst.engine,
                        ))
                    inst.sync_info = mybir.SyncInfo(
                        on_wait=waits[-cap:], on_update=list(si.on_update))
                new_insts.append(inst)
            bb.instructions = new_insts
    return ctr


def _plan(TA, TB):
    """Group blocks in pairs; per group two gather calls (even parity,
    odd parity), each covering the paired blocks' tiles contiguously.
    Returns (groups, SUMT): groups = list of dicts with
      blocks: tuple of block ids
      calls:  [(cls, tile_start, ntiles)] two entries
      seg:    {(block, cls): (tile_start, ntiles)}
    Tile indices are global (into dstrel / idx column space)."""
    groups = []
    toff = 0
    b = 0
    while b < NB:
        blocks = (b, b + 1) if b + 1 < NB else (b,)
        seg = {}
        callA_start = toff
        for blk in blocks:
            seg[(blk, 0)] = (toff, TA[blk])
            toff += TA[blk]
        callA_n = toff - callA_start
        callB_start = toff
        for blk in blocks:
            seg[(blk, 1)] = (toff, TB[blk])
            toff += TB[blk]
        callB_n = toff - callB_start
        groups.append({
            "blocks": blocks,
            "calls": [(0, callA_start, callA_n), (1, callB_start, callB_n)],
            "seg": seg,
        })
        b += 2
    return groups, toff


def build_kernel(TA, TB, reps=1, do_gather=True, do_compute=True):
    """TA/TB: per-block tile counts (len NB) for even/odd src parity,
    uniform across cores (max over cores, baked into the program).
    reps>1 replicates the whole 3-layer body (timing instrument only)."""
    groups, SUMT = _plan(TA, TB)

    nc = bass.Bass(num_swdge_queues=NQ)
    xT_in = nc.dram_tensor("xT", [IN_C, SHARD], F32, kind="ExternalInput")
    degbc_in = nc.dram_tensor("degbc", [HID, SHARD], F32, kind="ExternalInput")
    degnm_in = nc.dram_tensor("degnm", [128, NB], F32, kind="ExternalInput")
    idx_in = nc.dram_tensor("idx", [128, SUMT * 8], I16, kind="ExternalInput")
    dstrel_in = nc.dram_tensor("dstrel", [128, SUMT], F32, kind="ExternalInput")
    iota_in = nc.dram_tensor("iota", [128, 128], BF16, kind="ExternalInput")
    w1_in = nc.dram_tensor("w1", [IN_C, HID], F32, kind="ExternalInput")
    w2_in = nc.dram_tensor("w2", [HID, HID], F32, kind="ExternalInput")
    w3_in = nc.dram_tensor("w3", [HID, HID], F32, kind="ExternalInput")
    gb_in = nc.dram_tensor("gb", [HID, 6], F32, kind="ExternalInput")  # g1,be1,...
    out_t = nc.dram_tensor("outT", [HID, SHARD], F32, kind="ExternalOutput")

    # collective buffers
    ag_in = nc.dram_tensor("ag_in", [SHARD, HID], BF16)
    table = nc.dram_tensor("table", [NPAD, HID], BF16, addr_space="Shared")
    st_in = nc.dram_tensor("st_in", [HID, 2], F32)
    st_out = nc.dram_tensor("st_out", [HID, 2], F32, addr_space="Shared")

    rgroups = [list(range(NCORES))]

    with tile.TileContext(nc) as tc:
        with (
            tc.tile_pool(name="persist", bufs=1) as pp,
            tc.tile_pool(name="work", bufs=4) as wp,
            tc.tile_pool(name="small", bufs=2) as sp,
            tc.tile_pool(name="ohp", bufs=16) as ohpool,
            tc.tile_pool(name="psum", bufs=2, space="PSUM") as psp,
            tc.tile_pool(name="psum_sc", bufs=2, space="PSUM") as psc,
            tc.tile_pool(name="psum_tp", bufs=2, space="PSUM") as ptp,
        ):
            nc.gpsimd.load_library(library_config.mlp)

            # ---- persistent loads ----
            idx = pp.tile([128, SUMT * 8], I16)
            nc.sync.dma_start(idx[:], idx_in[:])
            dstrel = pp.tile([128, SUMT], F32)
            nc.sync.dma_start(dstrel[:], dstrel_in[:])
            iota_t = pp.tile([128, 128], BF16)
            nc.sync.dma_start(iota_t[:], iota_in[:])
            w1 = pp.tile([IN_C, HID], F32)
            nc.sync.dma_start(w1[:], w1_in[:])
            w2 = pp.tile([HID, HID], F32)
            nc.sync.dma_start(w2[:], w2_in[:])
            w3 = pp.tile([HID, HID], F32)
            nc.sync.dma_start(w3[:], w3_in[:])
            gb = pp.tile([HID, 6], F32)
            nc.sync.dma_start(gb[:], gb_in[:])
            xT = pp.tile([IN_C, SHARD], F32)
            nc.sync.dma_start(xT[:], xT_in[:])

            identf = pp.tile([128, 128], F32)
            make_identity(nc, identf[:])
            ident = pp.tile([128, 128], BF16)
            nc.vector.tensor_copy(ident[:], identf[:])
            eps_t = pp.tile([HID, 1], F32)
            nc.vector.memset(eps_t[:], float(BN_EPS))

            # dinv broadcast (channel-major): rsqrt(deg); deg_pad=1e30 -> ~0
            dinv_bc = pp.tile([HID, SHARD], F32)
            nc.sync.dma_start(dinv_bc[:], degbc_in[:])
            nc.scalar.sqrt(dinv_bc[:], dinv_bc[:])
            nc.vector.reciprocal(dinv_bc[:], dinv_bc[:])
            dinv_nm = pp.tile([128, NB], F32)
            nc.sync.dma_start(dinv_nm[:], degnm_in[:])
            nc.scalar.sqrt(dinv_nm[:], dinv_nm[:])
            nc.vector.reciprocal(dinv_nm[:], dinv_nm[:])

            # persistent activations
            S_t = pp.tile([HID, SHARD], F32)     # pre-BN conv output
            X_t = pp.tile([HID, SHARD], F32)     # post-BN/ReLU; also stat scratch
            H_f = pp.tile([HID, SHARD], F32)     # X @ W, channel-major
            nm_full = pp.tile([128, NB * HID], BF16)  # node-major table shard

            if not do_compute:
                nc.vector.memset(S_t[:], 0.0)

            nreg_cache = {}

            def nreg(v):
                if v not in nreg_cache:
                    nreg_cache[v] = nc.gpsimd.to_reg(v)
                return nreg_cache[v]

            table_pairs = table[:].rearrange("(q two) d -> q (two d)", two=2)

            for _rep in range(reps):
              for layer in range(3):
                  w = (w1, w2, w3)[layer]
                  kdim = IN_C if layer == 0 else HID
                  rhs = xT if layer == 0 else X_t

                  # ---- H = W^T @ X^T, channel-major f32 ----
                  col = 0
                  while col < SHARD:
                      nn = min(512, SHARD - col)
                      hp = psp.tile([HID, 512], F32, tag="wmm")
                      nc.tensor.matmul(hp[:, :nn],
                                       lhsT=w[:kdim, :],
                                       rhs=rhs[:kdim, col:col + nn],
                                       start=True, stop=True)
                      nc.vector.tensor_copy(H_f[:, col:col + nn], hp[:, :nn])
                      col += nn

                  # ---- table shard: transpose to node-major, scale, bf16 ----
                  for b in range(NB):
                      tp = ptp.tile([128, HID], F32, tag="tp")
                      nc.tensor.transpose(tp[:], H_f[:, b * 128:(b + 1) * 128],
                                          identf[0:HID, 0:HID])
                      nc.scalar.activation(nm_full[:, b * HID:(b + 1) * HID], tp[:],
                                           mybir.ActivationFunctionType.Identity,
                                           scale=dinv_nm[:, b:b + 1])
                      nc.sync.dma_start(ag_in[b * 128:(b + 1) * 128, :],
                                        nm_full[:, b * HID:(b + 1) * HID])

                  nc.gpsimd.collective_compute(
                      "AllGather", mybir.AluOpType.bypass, replica_groups=rgroups,
                      ins=[ag_in[:]], outs=[table[:]],
                  )

                  # ---- message passing, two gather calls per block pair ----
                  qn = 0
                  for grp in groups:
                      gstart = grp["calls"][0][1]
                      gtiles = sum(n for _, _, n in grp["calls"])
                      msg = wp.tile([128, gtiles * 128], BF16, tag="msg")
                      if not do_gather:
                          nc.vector.memset(msg[0:1, 0:2], 0.0)
                      if do_gather:
                          for _cls, tstart, ntile in grp["calls"]:
                              if ntile == 0:
                                  continue
                              rel = tstart - gstart
                              nc.gpsimd.dma_gather(
                                  out_ap=msg[:, rel * 128:(rel + ntile) * 128]
                                      .rearrange("p (n d) -> p n d", d=128),
                                  in_ap=table_pairs,
                                  idxs_ap=idx[:, tstart * 8:(tstart + ntile) * 8],
                                  num_idxs=ntile * 128, num_idxs_reg=nreg(ntile * 128),
                                  elem_size=128, single_packet=False, queue_num=qn % NQ,
                              )
                              qn += 1
                      for blk in grp["blocks"]:
                          if not do_compute:
                              continue
                          ps = psc.tile([HID, 128], F32, tag="scat")
                          # self loop: ps = (dinv*H)^T for this block
                          nc.tensor.matmul(ps[:],
                                           lhsT=nm_full[:, blk * HID:(blk + 1) * HID],
                                           rhs=ident[:],
                                           start=True, stop=False)
                          segs = [(0, grp["seg"][(blk, 0)]), (1, grp["seg"][(blk, 1)])]
                          ntot = sum(n for _, (_, n) in segs)
                          ti = 0
                          for cls, (tstart, ntile) in segs:
                              for t in range(ntile):
                                  gcol = tstart + t
                                  rel = gcol - gstart
                                  oh = ohpool.tile([128, 128], BF16, tag="oh")
                                  nc.vector.tensor_scalar(
                                      out=oh[:], in0=iota_t[:],
                                      scalar1=dstrel[:, gcol:gcol + 1], scalar2=None,
                                      op0=mybir.AluOpType.is_equal)
                                  off = rel * 128 + cls * HID
                                  nc.tensor.matmul(ps[:],
                                                   lhsT=msg[:, off:off + HID],
                                                   rhs=oh[:],
                                                   start=False, stop=(ti == ntot - 1))
                                  ti += 1
                          nc.vector.tensor_tensor(
                              out=S_t[:, blk * 128:(blk + 1) * 128], in0=ps[:],
                              in1=dinv_bc[:, blk * 128:(blk + 1) * 128],
                              op=mybir.AluOpType.mult)

                  # ---- BN stats (local) ----
                  sums = sp.tile([HID, 2], F32, tag="sums")
                  nc.scalar.activation(X_t[:], S_t[:], mybir.ActivationFunctionType.Identity,
                                       accum_out=sums[:, 0:1])
                  nc.scalar.activation(X_t[:], S_t[:], mybir.ActivationFunctionType.Square,
                                       accum_out=sums[:, 1:2])
                  nc.sync.dma_start(st_in[:], sums[:])
                  nc.gpsimd.collective_compute(
                      "AllReduce", mybir.AluOpType.add, replica_groups=rgroups,
                      ins=[st_in[:]], outs=[st_out[:]],
                  )
                  gsums = sp.tile([HID, 2], F32, tag="gsums")
                  nc.sync.dma_start(gsums[:], st_out[:])

                  # mean/var -> A = g*rsqrt(var+eps), B = be - mean*A
                  stat = sp.tile([HID, 4], F32, tag="stat")
                  nc.vector.tensor_scalar(out=stat[:, 0:2], in0=gsums[:], scalar1=1.0 / N,
                                          scalar2=None, op0=mybir.AluOpType.mult)
                  nc.vector.tensor_tensor(out=stat[:, 2:3], in0=stat[:, 0:1],
                                          in1=stat[:, 0:1], op=mybir.AluOpType.mult)
                  nc.vector.tensor_tensor(out=stat[:, 2:3], in0=stat[:, 1:2],
                                          in1=stat[:, 2:3], op=mybir.AluOpType.subtract)
                  nc.scalar.activation(stat[:, 3:4], stat[:, 2:3],
                                       mybir.ActivationFunctionType.Sqrt, bias=eps_t[:, 0:1])
                  nc.vector.reciprocal(stat[:, 3:4], stat[:, 3:4])
                  ab = sp.tile([HID, 2], F32, tag="ab")
                  nc.vector.tensor_tensor(out=ab[:, 0:1], in0=stat[:, 3:4],
                                          in1=gb[:, 2 * layer:2 * layer + 1],
                                          op=mybir.AluOpType.mult)
                  nc.vector.tensor_tensor(out=ab[:, 1:2], in0=stat[:, 0:1],
                                          in1=ab[:, 0:1], op=mybir.AluOpType.mult)
                  nc.vector.tensor_tensor(out=ab[:, 1:2],
                                          in0=gb[:, 2 * layer + 1:2 * layer + 2],
                                          in1=ab[:, 1:2], op=mybir.AluOpType.subtract)
                  # X = Relu(S*A + B)
                  nc.scalar.activation(X_t[:], S_t[:], mybir.ActivationFunctionType.Relu,
                                       bias=ab[:, 1:2], scale=ab[:, 0:1])

            nc.sync.dma_start(out_t[:], X_t[:])

    if _CLEAN[0]:
        _clean_waits(nc)
    _split_multi_waits(nc)
    lower_extended_insts(nc)
    return nc


def _prep(x, edge_index):
    """Host-side integer preprocessing: shard / sort / pad the edge list.
    Self loops are NOT added as edges (folded in-kernel); deg includes them."""
    src = np.asarray(edge_index[0], dtype=np.int64)
    dst = np.asarray(edge_index[1], dtype=np.int64)
    deg = np.bincount(dst, minlength=NPAD).astype(np.float32) + 1.0
    deg[N:] = 1e30  # kill padding rows via dinv ~ 0

    order = np.argsort(dst, kind="stable")
    src, dst = src[order], dst[order]
    blk = (dst // 128).astype(np.int64)
    # edges grouped per global block; within block split by src parity
    counts = {}
    seg = {}
    bstart = np.searchsorted(blk, np.arange(NPAD // 128 + 1))
    for gb in range(NPAD // 128):
        s, e = bstart[gb], bstart[gb + 1]
        bs, bd = src[s:e], dst[s:e]
        a_mask = (bs % 2) == 0
        seg[gb] = (bs[a_mask], bd[a_mask], bs[~a_mask], bd[~a_mask])
        counts[gb] = (a_mask.sum(), (~a_mask).sum())

    TA = [0] * NB
    TB = [0] * NB
    for gb in range(NPAD // 128):
        bloc = gb % NB
        ca, cb = counts[gb]
        TA[bloc] = max(TA[bloc], -(-int(ca) // 128))
        TB[bloc] = max(TB[bloc], -(-int(cb) // 128))
    TA = [max(t, 1) for t in TA]
    TB = [max(t, 1) for t in TB]

    groups, SUMT = _plan(TA, TB)
    idx_all = np.zeros((NCORES, 128, SUMT * 8), dtype=np.int16)
    dre_all = np.full((NCORES, 128, SUMT), -1.0, dtype=np.float32)
    for c in range(NCORES):
        for grp in groups:
            for blk_ in grp["blocks"]:
                gb = c * NB + blk_
                sa, da, sb, db = seg[gb]
                for cls, (ss, dd) in ((0, (sa, da)), (1, (sb, db))):
                    tstart, T = grp["seg"][(blk_, cls)]
                    nslots = T * 128
                    sl_idx = np.zeros(nslots, dtype=np.int16)
                    sl_dre = np.full(nslots, -1.0, dtype=np.float32)
                    k = len(ss)
                    sl_idx[:k] = (ss >> 1).astype(np.int16)
                    sl_dre[:k] = (dd - gb * 128).astype(np.float32)
                    wr = sl_idx.reshape(nslots // 16, 16).T
                    idx_all[c, :, tstart * 8:(tstart + T) * 8] = np.tile(wr, (8, 1))
                    dre_all[c, :, tstart:tstart + T] = sl_dre.reshape(T, 128).T
    return deg, TA, TB, idx_all, dre_all


_CACHE = {}
_REPS = [1]


def build_and_maps(x, edge_index, w1, b1, g1, be1, w2, b2, g2, be2, w3, b3, g3, be3):
    x = np.asarray(x, dtype=np.float32)
    deg, TA, TB, idx_all, dre_all = _prep(x, edge_index)

    key = (tuple(TA), tuple(TB), _REPS[0])
    if key not in _CACHE:
        _CACHE[key] = build_kernel(TA, TB, reps=_REPS[0])
    nc = _CACHE[key]

    bf16 = mybir.dt.np(BF16)
    xpad = np.zeros((NPAD, IN_C), dtype=np.float32)
    xpad[:N] = x
    iota = np.broadcast_to(np.arange(128, dtype=np.float32), (128, 128)).astype(bf16)
    in_maps = []
    for c in range(NCORES):
        sl = slice(c * SHARD, (c + 1) * SHARD)
        deg_c = deg[sl]
        in_maps.append({
            "xT": np.ascontiguousarray(xpad[sl].T),
            "degbc": np.ascontiguousarray(np.broadcast_to(deg_c, (HID, SHARD))),
            "degnm": np.ascontiguousarray(deg_c.reshape(NB, 128).T),
            "idx": idx_all[c],
            "dstrel": dre_all[c],
            "iota": np.ascontiguousarray(iota),
            "w1": np.asarray(w1, dtype=np.float32),
            "w2": np.asarray(w2, dtype=np.float32),
            "w3": np.asarray(w3, dtype=np.float32),
            "gb": np.stack([np.asarray(a, dtype=np.float32)
                            for a in (g1, be1, g2, be2, g3, be3)], axis=1),
        })

    return nc, in_maps


def kernel(**inputs):
    nc, in_maps = build_and_maps(**inputs)
    from concourse.bass_utils import run_bass_kernel_spmd
    res = run_bass_kernel_spmd(nc, in_maps, list(range(NCORES)))
    out = np.concatenate([res.results[c]["outT"].T for c in range(NCORES)], axis=0)
    return np.ascontiguousarray(out[:N])
